# revision 32
# baseline (speedup 1.0000x reference)
# Bass/Trainium2 kernel for nn_Attention (Bahdanau-style attention scores).
#
# reference math (per batch b):
#   e_proj[s, o] = sum_e enc[b, s, e] * We[o, e]          (We = attn_W[:, H:])
#   h_proj[o]    = sum_e hidden[b, e] * Wh[o, e]          (Wh = attn_W[:, :H])
#   energy       = tanh(e_proj + h_proj + attn_b)
#   scores[s]    = sum_o energy[s, o] * v[o]
#   out[b]       = softmax(scores)
#
# Strategy (8 NeuronCores, data-parallel over batch, 4 batches/core):
#   - SWDGE (gpsimd) DMA loads encoder slices HBM->SBUF with fp32->fp16 cast
#     in flight.
#   - XBAR (HWDGE) DMA transposes [128,128] fp16 tiles SBUF->SBUF to put the
#     contraction dim (e) on partitions.
#   - TensorE computes e_proj TRANSPOSED: psum[o_chunk, rows] = WeT.T @ encT,
#     so the (h_proj + attn_b) add becomes a per-partition bias fused into
#     the ScalarE tanh activation, and the v-dot becomes 4 more matmuls.
#   - Softmax tail on [4, 2048] scores via DVE/ACT.
import os

import numpy as np

import concourse.bass as bass
import concourse.mybir as mybir
import concourse.tile as tile
from concourse import bacc
from concourse.bass_utils import run_bass_kernel_spmd
from concourse.masks import make_identity

H = 512          # hidden dim / output dim of attn matmul
E = 2 * H        # encoder feature dim (1024)
B = 32           # global batch
S = 2048         # sequence length
NCORES = 8
BL = B // NCORES  # batches per core (4)

RB = 512         # rows (s positions) per block
RT = RB // 128   # 128-row subtiles per block (4)
NBLK = S // RB   # blocks per batch (4)
EC = E // 128    # e chunks (8)
OC = H // 128    # o chunks (4)

F32 = mybir.dt.float32
MMDT = mybir.dt.float16      # matmul operand dtype
NP_MMDT = np.float16

ActFn = mybir.ActivationFunctionType


def build_nc():
    nc = bacc.Bacc(
        "TRN2",
        target_bir_lowering=False,
        debug=False,
        enable_asserts=False,
        num_devices=NCORES,
    )

    enc = nc.dram_tensor("enc", [BL, S, E], F32, kind="ExternalInput").ap()
    # host-prearranged small tensors (already in SBUF layout):
    weT_l = nc.dram_tensor("weT_l", [128, EC, H], MMDT, kind="ExternalInput").ap()
    whT_l = nc.dram_tensor("whT_l", [128, OC, H], F32, kind="ExternalInput").ap()
    hT_l = nc.dram_tensor("hT_l", [128, OC, BL], F32, kind="ExternalInput").ap()
    v_l = nc.dram_tensor("v_l", [128, OC, 1], MMDT, kind="ExternalInput").ap()
    ab_l = nc.dram_tensor("ab_l", [128, OC], F32, kind="ExternalInput").ap()
    out = nc.dram_tensor("out", [BL, S], F32, kind="ExternalOutput").ap()

    with tile.TileContext(nc) as tc:
        with (
            tc.tile_pool(name="const", bufs=1) as const_pool,
            tc.tile_pool(name="enc_in", bufs=3) as enc_pool,
            tc.tile_pool(name="encT", bufs=3) as tr_pool,
            tc.tile_pool(name="energy", bufs=3) as en_pool,
            tc.tile_pool(name="scores", bufs=2) as sc_pool,
            tc.tile_pool(name="small", bufs=2) as small_pool,
            tc.tile_pool(name="psumT", bufs=3, space="PSUM") as psum_pool,
            tc.tile_pool(name="ttps", bufs=4, space="PSUM") as psum_t_pool,
            tc.tile_pool(name="psum_s", bufs=1, space="PSUM") as psum_s_pool,
        ):
            # ---- setup ----
            # identity (gpsimd compute) first so transposes aren't gated;
            # const loads on SyncE (idle) so the gpsimd queue starts on the
            # first encoder block immediately.
            ident = const_pool.tile([128, 128], MMDT)
            make_identity(nc, ident[:])
            we_sb = const_pool.tile([128, EC, H], MMDT)
            nc.sync.dma_start(we_sb[:], weT_l)
            wh_sb = const_pool.tile([128, OC, H], F32)
            nc.sync.dma_start(wh_sb[:], whT_l)
            hT_sb = const_pool.tile([128, OC, BL], F32)
            nc.sync.dma_start(hT_sb[:], hT_l)
            v_sb = const_pool.tile([128, OC, 1], MMDT)
            nc.sync.dma_start(v_sb[:], v_l)
            ab_sb = const_pool.tile([128, OC], F32)
            nc.sync.dma_start(ab_sb[:], ab_l)

            # HAM warmup: dummy matmuls depending only on `ident` (ready
            # almost immediately), so the PE clock-gate (4/8 cold) releases
            # before real work arrives and nothing queues behind slow loads.
            for w in range(24):
                wp = psum_pool.tile([128, RB], F32, tag="psumT")
                nc.tensor.matmul(
                    wp[:, 0:128], lhsT=ident[:], rhs=ident[:], start=True,
                    stop=True,
                )

            # ---- hb[o, b] = h_proj[o, b] + attn_b[o], laid out [128, OC, BL]
            # (emitted after block-0's transposes so the in-order PE stream
            # doesn't stall on the const loads)
            hb_sb = const_pool.tile([128, OC, BL], F32)

            def emit_hb():
                for oc in range(OC):
                    ph = psum_pool.tile([128, BL], F32, tag="psumT")
                    for ec in range(OC):  # contraction over H: OC chunks
                        nc.tensor.matmul(
                            ph[:],
                            lhsT=wh_sb[:, ec, oc * 128:(oc + 1) * 128],
                            rhs=hT_sb[:, ec, :],
                            start=(ec == 0),
                            stop=(ec == OC - 1),
                        )
                    nc.vector.tensor_scalar_add(
                        hb_sb[:, oc, :], ph[:], ab_sb[:, oc:oc + 1]
                    )

            # ---- main loop over (batch, block) ----
            XBAR_EC = 0   # e-chunks transposed via XBAR DMA (on idle SyncE)
            for b in range(BL):
                # exp(scores) accumulated per block; no max subtraction needed
                # (energy in (-1,1) and |v|_1 ~ 11 bound |scores| far inside
                # fp32 exp range)
                exb = sc_pool.tile([1, S], F32, tag="ex")
                psums_b = small_pool.tile([1, NBLK], F32, tag="psum_part")
                for blk in range(NBLK):
                    s0 = blk * RB
                    # load 512 rows x 1024 e, casting fp32 -> fp16 in the
                    # DMA; rt-granular so transposes start after 512KB
                    et = enc_pool.tile([128, RT, E], MMDT, tag="et")
                    for rt in range(RT):
                        nc.gpsimd.dma_start(
                            et[:, rt, :],
                            enc[b, s0 + rt * 128:s0 + (rt + 1) * 128, :],
                        )
                    # transpose to [e, rows] layout: a couple of e-chunks on
                    # the XBAR (SyncE is otherwise idle), the rest as PE
                    # transpose-matmuls evacuated PSUM->SBUF via DVE/ACT
                    tt = tr_pool.tile([128, EC, RB], MMDT, tag="tt")
                    for ec in range(EC):
                        if ec < XBAR_EC:
                            for rt in range(RT):
                                nc.sync.dma_start(
                                    tt[:, ec, rt * 128:(rt + 1) * 128],
                                    et[:, rt, ec * 128:(ec + 1) * 128],
                                    transpose=True,
                                )
                            continue
                        tp = psum_t_pool.tile([128, RB], MMDT, tag="ttps")
                        for rt in range(RT):
                            nc.tensor.transpose(
                                tp[:, rt * 128:(rt + 1) * 128],
                                et[:, rt, ec * 128:(ec + 1) * 128],
                                ident[:],
                            )
                        if ec < 5:
                            nc.vector.tensor_copy(tt[:, ec, :], tp[:])
                        else:
                            nc.scalar.copy(tt[:, ec, :], tp[:])
                    if b == 0 and blk == 0:
                        emit_hb()
                    # main matmuls: psum[o_chunk 128, rows 512] += WeT.T @ encT
                    en = en_pool.tile([128, OC, RB], MMDT, tag="en")
                    for oc in range(OC):
                        pe_t = psum_pool.tile([128, RB], F32, tag="psumT")
                        for ec in range(EC):
                            nc.tensor.matmul(
                                pe_t[:],
                                lhsT=we_sb[:, ec, oc * 128:(oc + 1) * 128],
                                rhs=tt[:, ec, :],
                                start=(ec == 0),
                                stop=(ec == EC - 1),
                            )
                        # energy = tanh(psum + hb) fused via per-partition bias
                        nc.scalar.activation(
                            en[:, oc, :],
                            pe_t[:],
                            ActFn.Tanh,
                            bias=hb_sb[:, oc, b:b + 1],
                        )
                    # scores: psum[1, rows] = v.T @ energy
                    ps = psum_s_pool.tile([1, RB], F32, tag="psum_s")
                    for oc in range(OC):
                        nc.tensor.matmul(
                            ps[:],
                            lhsT=v_sb[:, oc, :],
                            rhs=en[:, oc, :],
                            start=(oc == 0),
                            stop=(oc == OC - 1),
                        )
                    # exp straight from PSUM with fused partial-sum accum
                    nc.scalar.activation(
                        exb[0:1, s0:s0 + RB], ps[:], ActFn.Exp,
                        accum_out=psums_b[0:1, blk:blk + 1],
                    )
                # ---- per-batch normalize (overlaps later batches' compute) --
                smb = small_pool.tile([1, 1], F32, tag="sm")
                nc.vector.reduce_sum(
                    smb[:], psums_b[:], axis=mybir.AxisListType.X
                )
                rcb = small_pool.tile([1, 1], F32, tag="rc")
                nc.vector.reciprocal(rcb[:], smb[:])
                outb = sc_pool.tile([1, S], F32, tag="outp")
                nc.vector.tensor_scalar_mul(outb[:], exb[:], rcb[:])
                nc.sync.dma_start(out[b:b + 1, :], outb[:, :])

    nc.compile()
    return nc


def _prep_host_inputs(hidden, encoder_outputs, attn_W, attn_b, v_W):
    """Build per-core input maps. Small tensors are pre-arranged into their
    SBUF layouts host-side; the big encoder tensor is just sliced."""
    Wh = attn_W[:, :H]                      # [H, H]  (o, e)
    We = attn_W[:, H:]                      # [H, 2H] (o, e)
    # weT_l[p, ec, o] = We[o, ec*128+p]
    weT_l = np.ascontiguousarray(
        We.T.reshape(EC, 128, H).transpose(1, 0, 2)
    ).astype(NP_MMDT)
    whT_l = np.ascontiguousarray(
        Wh.T.reshape(OC, 128, H).transpose(1, 0, 2)
    ).astype(np.float32)
    # v_l[p, oc, 0] = v[oc*128+p]
    v = v_W[0]
    v_l = np.ascontiguousarray(
        v.reshape(OC, 128, 1).transpose(1, 0, 2)
    ).astype(NP_MMDT)
    ab_l = np.ascontiguousarray(attn_b.reshape(OC, 128).T).astype(np.float32)

    hiddenT = hidden.T                      # [H, B]
    in_maps = []
    for c in range(NCORES):
        bsl = slice(c * BL, (c + 1) * BL)
        hT_slice = hiddenT[:, bsl]          # [H, BL]
        hT_l = np.ascontiguousarray(
            hT_slice.reshape(OC, 128, BL).transpose(1, 0, 2)
        ).astype(np.float32)
        in_maps.append({
            "enc": np.ascontiguousarray(encoder_outputs[bsl]),
            "weT_l": weT_l,
            "whT_l": whT_l,
            "hT_l": hT_l,
            "v_l": v_l,
            "ab_l": ab_l,
        })
    return in_maps


_NC_CACHE = {}


def kernel(hidden, encoder_outputs, attn_W, attn_b, v_W):
    in_maps = _prep_host_inputs(
        np.asarray(hidden, dtype=np.float32),
        np.asarray(encoder_outputs, dtype=np.float32),
        np.asarray(attn_W, dtype=np.float32),
        np.asarray(attn_b, dtype=np.float32),
        np.asarray(v_W, dtype=np.float32),
    )
    if "nc" not in _NC_CACHE:
        _NC_CACHE["nc"] = build_nc()
    nc = _NC_CACHE["nc"]

    trace = bool(int(os.environ.get("BASSK_TRACE", "0")))
    res = run_bass_kernel_spmd(
        nc, in_maps, core_ids=list(range(NCORES)), trace=trace
    )
    if trace and res.exec_time_ns is not None:
        print(f"HW exec time: {res.exec_time_ns} ns")
        if res.instructions_and_trace is not None:
            print(f"trace: {res.instructions_and_trace[1]}")
    out = np.concatenate([r["out"] for r in res.results], axis=0)
    return out.astype(np.float32)


# revision 38
# speedup vs baseline: 1.2583x; 1.2583x over previous
# Bass/Trainium2 kernel for nn_Attention (Bahdanau-style attention scores).
#
# reference math (per batch b):
#   e_proj[s, o] = sum_e enc[b, s, e] * We[o, e]          (We = attn_W[:, H:])
#   h_proj[o]    = sum_e hidden[b, e] * Wh[o, e]          (Wh = attn_W[:, :H])
#   energy       = tanh(e_proj + h_proj + attn_b)
#   scores[s]    = sum_o energy[s, o] * v[o]
#   out[b]       = softmax(scores)
#
# Strategy (8 NeuronCores, data-parallel over batch, 4 batches/core):
#   - SWDGE (gpsimd) DMA loads encoder slices HBM->SBUF with fp32->fp16 cast
#     in flight.
#   - XBAR (HWDGE) DMA transposes [128,128] fp16 tiles SBUF->SBUF to put the
#     contraction dim (e) on partitions.
#   - TensorE computes e_proj TRANSPOSED: psum[o_chunk, rows] = WeT.T @ encT,
#     so the (h_proj + attn_b) add becomes a per-partition bias fused into
#     the ScalarE tanh activation, and the v-dot becomes 4 more matmuls.
#   - Softmax tail on [4, 2048] scores via DVE/ACT.
import os

import numpy as np

import concourse.bass as bass
import concourse.mybir as mybir
import concourse.tile as tile
from concourse import bacc
from concourse.bass_utils import run_bass_kernel_spmd
from concourse.masks import make_identity

H = 512          # hidden dim / output dim of attn matmul
E = 2 * H        # encoder feature dim (1024)
B = 32           # global batch
S = 2048         # sequence length
NCORES = 8
BL = B // NCORES  # batches per core (4)

RB = 512         # rows (s positions) per block
RT = RB // 128   # 128-row subtiles per block (4)
NBLK = S // RB   # blocks per batch (4)
EC = E // 128    # e chunks (8)
OC = H // 128    # o chunks (4)

F32 = mybir.dt.float32
MMDT = mybir.dt.float16      # matmul operand dtype
NP_MMDT = np.float16

ActFn = mybir.ActivationFunctionType


def build_nc():
    nc = bacc.Bacc(
        "TRN2",
        target_bir_lowering=False,
        debug=False,
        enable_asserts=False,
        num_devices=NCORES,
    )

    enc = nc.dram_tensor("enc", [BL, S, E], F32, kind="ExternalInput").ap()
    # host-prearranged small tensors (already in SBUF layout):
    weT_l = nc.dram_tensor("weT_l", [128, EC, H], MMDT, kind="ExternalInput").ap()
    whT_l = nc.dram_tensor("whT_l", [128, OC, H], MMDT, kind="ExternalInput").ap()
    hT_l = nc.dram_tensor("hT_l", [128, OC, BL], MMDT, kind="ExternalInput").ap()
    v_l = nc.dram_tensor("v_l", [128, OC, 1], MMDT, kind="ExternalInput").ap()
    ab_l = nc.dram_tensor("ab_l", [128, OC], F32, kind="ExternalInput").ap()
    out = nc.dram_tensor("out", [BL, S], F32, kind="ExternalOutput").ap()

    with tile.TileContext(nc) as tc:
        with (
            tc.tile_pool(name="const", bufs=1) as const_pool,
            tc.tile_pool(name="enc_in", bufs=3) as enc_pool,
            tc.tile_pool(name="encT", bufs=3) as tr_pool,
            tc.tile_pool(name="energy", bufs=3) as en_pool,
            tc.tile_pool(name="scores", bufs=2) as sc_pool,
            tc.tile_pool(name="small", bufs=2) as small_pool,
            tc.tile_pool(name="psumT", bufs=3, space="PSUM") as psum_pool,
            tc.tile_pool(name="ttps", bufs=4, space="PSUM") as psum_t_pool,
            tc.tile_pool(name="psum_s", bufs=1, space="PSUM") as psum_s_pool,
        ):
            # ---- setup ----
            # identity (gpsimd compute) first so transposes aren't gated;
            # const loads on SyncE (idle) so the gpsimd queue starts on the
            # first encoder block immediately.
            ident = const_pool.tile([128, 128], MMDT)
            make_identity(nc, ident[:])
            # small hb inputs early on the gpsimd queue (ahead of block
            # loads) so the hb matmuls never stall the in-order PE stream
            wh_sb = const_pool.tile([128, OC, H], MMDT)
            nc.gpsimd.dma_start(wh_sb[:], whT_l)
            hT_sb = const_pool.tile([128, OC, BL], MMDT)
            nc.gpsimd.dma_start(hT_sb[:], hT_l)
            ab_sb = const_pool.tile([128, OC], F32)
            nc.gpsimd.dma_start(ab_sb[:], ab_l)
            we_sb = const_pool.tile([128, EC, H], MMDT)
            nc.sync.dma_start(we_sb[:], weT_l)
            v_sb = const_pool.tile([128, OC, 1], MMDT)
            nc.sync.dma_start(v_sb[:], v_l)

            # HAM warmup: dummy matmuls depending only on `ident` (ready
            # almost immediately), so the PE clock-gate (4/8 cold) releases
            # before real work arrives and nothing queues behind slow loads.
            for w in range(24):
                wp = psum_pool.tile([128, RB], F32, tag="psumT")
                nc.tensor.matmul(
                    wp[:, 0:128], lhsT=ident[:], rhs=ident[:], start=True,
                    stop=True,
                )

            # ---- hb[o, b] = h_proj[o, b] + attn_b[o], laid out [128, OC, BL]
            hb_sb = const_pool.tile([128, OC, BL], F32)
            for oc in range(OC):
                ph = psum_pool.tile([128, BL], F32, tag="psumT")
                for ec in range(OC):  # contraction over H: OC chunks of 128
                    nc.tensor.matmul(
                        ph[:],
                        lhsT=wh_sb[:, ec, oc * 128:(oc + 1) * 128],
                        rhs=hT_sb[:, ec, :],
                        start=(ec == 0),
                        stop=(ec == OC - 1),
                    )
                nc.vector.tensor_scalar_add(
                    hb_sb[:, oc, :], ph[:], ab_sb[:, oc:oc + 1]
                )

            # ---- main loop over (batch, block) ----
            XBAR_EC = 0   # e-chunks transposed via XBAR DMA (on idle SyncE)
            for b in range(BL):
                # exp(scores) accumulated per block; no max subtraction needed
                # (energy in (-1,1) and |v|_1 ~ 11 bound |scores| far inside
                # fp32 exp range)
                exb = sc_pool.tile([1, S], F32, tag="ex")
                psums_b = small_pool.tile([1, NBLK], F32, tag="psum_part")
                for blk in range(NBLK):
                    s0 = blk * RB
                    # load 512 rows x 1024 e, casting fp32 -> fp16 in the
                    # DMA; rt-granular so transposes start after 512KB
                    et = enc_pool.tile([128, RT, E], MMDT, tag="et")
                    for rt in range(RT):
                        nc.gpsimd.dma_start(
                            et[:, rt, :],
                            enc[b, s0 + rt * 128:s0 + (rt + 1) * 128, :],
                        )
                    # transpose to [e, rows] layout: a couple of e-chunks on
                    # the XBAR (SyncE is otherwise idle), the rest as PE
                    # transpose-matmuls evacuated PSUM->SBUF via DVE/ACT
                    tt = tr_pool.tile([128, EC, RB], MMDT, tag="tt")
                    for ec in range(EC):
                        if ec < XBAR_EC:
                            for rt in range(RT):
                                nc.sync.dma_start(
                                    tt[:, ec, rt * 128:(rt + 1) * 128],
                                    et[:, rt, ec * 128:(ec + 1) * 128],
                                    transpose=True,
                                )
                            continue
                        tp = psum_t_pool.tile([128, RB], MMDT, tag="ttps")
                        for rt in range(RT):
                            nc.tensor.transpose(
                                tp[:, rt * 128:(rt + 1) * 128],
                                et[:, rt, ec * 128:(ec + 1) * 128],
                                ident[:],
                            )
                        if ec < 5:
                            nc.vector.tensor_copy(tt[:, ec, :], tp[:])
                        else:
                            nc.scalar.copy(tt[:, ec, :], tp[:])
                    # main matmuls: psum[o_chunk 128, rows 512] += WeT.T @ encT
                    en = en_pool.tile([128, OC, RB], MMDT, tag="en")
                    for oc in range(OC):
                        pe_t = psum_pool.tile([128, RB], F32, tag="psumT")
                        for ec in range(EC):
                            nc.tensor.matmul(
                                pe_t[:],
                                lhsT=we_sb[:, ec, oc * 128:(oc + 1) * 128],
                                rhs=tt[:, ec, :],
                                start=(ec == 0),
                                stop=(ec == EC - 1),
                            )
                        # energy = tanh(psum + hb) fused via per-partition bias
                        nc.scalar.activation(
                            en[:, oc, :],
                            pe_t[:],
                            ActFn.Tanh,
                            bias=hb_sb[:, oc, b:b + 1],
                        )
                    # scores: psum[1, rows] = v.T @ energy
                    ps = psum_s_pool.tile([1, RB], F32, tag="psum_s")
                    for oc in range(OC):
                        nc.tensor.matmul(
                            ps[:],
                            lhsT=v_sb[:, oc, :],
                            rhs=en[:, oc, :],
                            start=(oc == 0),
                            stop=(oc == OC - 1),
                        )
                    # exp straight from PSUM with fused partial-sum accum
                    nc.scalar.activation(
                        exb[0:1, s0:s0 + RB], ps[:], ActFn.Exp,
                        accum_out=psums_b[0:1, blk:blk + 1],
                    )
                # ---- per-batch normalize (overlaps later batches' compute) --
                smb = small_pool.tile([1, 1], F32, tag="sm")
                nc.vector.reduce_sum(
                    smb[:], psums_b[:], axis=mybir.AxisListType.X
                )
                rcb = small_pool.tile([1, 1], F32, tag="rc")
                nc.vector.reciprocal(rcb[:], smb[:])
                outb = sc_pool.tile([1, S], F32, tag="outp")
                nc.vector.tensor_scalar_mul(outb[:], exb[:], rcb[:])
                nc.sync.dma_start(out[b:b + 1, :], outb[:, :])

    nc.compile()
    return nc


def _prep_host_inputs(hidden, encoder_outputs, attn_W, attn_b, v_W):
    """Build per-core input maps. Small tensors are pre-arranged into their
    SBUF layouts host-side; the big encoder tensor is just sliced."""
    Wh = attn_W[:, :H]                      # [H, H]  (o, e)
    We = attn_W[:, H:]                      # [H, 2H] (o, e)
    # weT_l[p, ec, o] = We[o, ec*128+p]
    weT_l = np.ascontiguousarray(
        We.T.reshape(EC, 128, H).transpose(1, 0, 2)
    ).astype(NP_MMDT)
    whT_l = np.ascontiguousarray(
        Wh.T.reshape(OC, 128, H).transpose(1, 0, 2)
    ).astype(NP_MMDT)
    # v_l[p, oc, 0] = v[oc*128+p]
    v = v_W[0]
    v_l = np.ascontiguousarray(
        v.reshape(OC, 128, 1).transpose(1, 0, 2)
    ).astype(NP_MMDT)
    ab_l = np.ascontiguousarray(attn_b.reshape(OC, 128).T).astype(np.float32)

    hiddenT = hidden.T                      # [H, B]
    in_maps = []
    for c in range(NCORES):
        bsl = slice(c * BL, (c + 1) * BL)
        hT_slice = hiddenT[:, bsl]          # [H, BL]
        hT_l = np.ascontiguousarray(
            hT_slice.reshape(OC, 128, BL).transpose(1, 0, 2)
        ).astype(NP_MMDT)
        in_maps.append({
            "enc": np.ascontiguousarray(encoder_outputs[bsl]),
            "weT_l": weT_l,
            "whT_l": whT_l,
            "hT_l": hT_l,
            "v_l": v_l,
            "ab_l": ab_l,
        })
    return in_maps


_NC_CACHE = {}


def kernel(hidden, encoder_outputs, attn_W, attn_b, v_W):
    in_maps = _prep_host_inputs(
        np.asarray(hidden, dtype=np.float32),
        np.asarray(encoder_outputs, dtype=np.float32),
        np.asarray(attn_W, dtype=np.float32),
        np.asarray(attn_b, dtype=np.float32),
        np.asarray(v_W, dtype=np.float32),
    )
    if "nc" not in _NC_CACHE:
        _NC_CACHE["nc"] = build_nc()
    nc = _NC_CACHE["nc"]

    trace = bool(int(os.environ.get("BASSK_TRACE", "0")))
    res = run_bass_kernel_spmd(
        nc, in_maps, core_ids=list(range(NCORES)), trace=trace
    )
    if trace and res.exec_time_ns is not None:
        print(f"HW exec time: {res.exec_time_ns} ns")
        if res.instructions_and_trace is not None:
            print(f"trace: {res.instructions_and_trace[1]}")
    out = np.concatenate([r["out"] for r in res.results], axis=0)
    return out.astype(np.float32)
